# revision 2
# baseline (speedup 1.0000x reference)
"""Trainium2 Bass kernel for the Keras SimpleRNN wrapper — v7.

    xproj = inputs @ (s*Wx) + s*b              # [B, T, H]  (held as s*xproj)
    h_t   = tanh((xproj_t + h_{t-1} @ (s*Wh)) / s)
    y     = h @ Wo + bo                        # [B, T, O]

Data-parallel over batch (8 rows/core). v3 changes vs v2:
  - FULL UNROLL: no Fori hardware loops -> pure sequential instruction
    streams (no backward-branch instruction-queue refetch stalls), and
    every AP/semaphore wait is a static constant.
  - tanh writes h directly into a 128-slot SBUF ring (HS); the y matmuls
    read the ring with strided APs. The per-step DVE copy, the CS
    semaphore and the per-block PE barrier on DVE are gone.
  - phase 1 uses a 4-deep PSUM ring with bias-adds alternating between
    the Scalar (Act) and Vector engines, so the PE never waits for PSUM.
  - optional fp8e3 (e3m4) Wh with power-of-2 prescale s folded into
    Wx/b on the host and tanh(in * 1/s) on the Act; fp8 halves the
    LDWEIGHTS time per step matmul (FWL is byte-bound).
"""

import numpy as np
import ml_dtypes

import concourse.bass as bass
import concourse.mybir as mybir
from concourse import bacc
from concourse.bass_utils import run_bass_kernel_spmd

BF16 = mybir.dt.bfloat16
FP32 = mybir.dt.float32
E3 = mybir.dt.float8e3
bfnp = ml_dtypes.bfloat16
e3np = ml_dtypes.float8_e3m4

B, D, H, O = 64, 256, 1024, 128
NCORES = 8
BL = B // NCORES          # 8 batch rows per core
SLOT = 8 * BL             # 64 cols per timestep slot
KH = H // 128             # 8 contraction chunks for Wh/Wo
MH = H // 128
KD = D // 128             # 2 contraction chunks for Wx
RING = 128                # h-ring slots (2 blocks)

# --- tunables (set before first _get_nc call) ---
WH_FP8 = False            # fp8 is a dead end: LDWEIGHTS is col-count-bound
SCALE = 16.0              # power-of-2 prescale (only used when WH_FP8)

_cached = {}
LAST_RESULTS = None


def _build(T=512):
    NTOK = BL * T
    NT = 512              # tokens per phase-1/3 tile (64 steps x 8 batch)
    NBLK = T // 64
    WHT = E3 if WH_FP8 else BF16

    nc = bacc.Bacc("TRN2", target_bir_lowering=False, debug=False)

    xT = nc.dram_tensor("xT", [D, NTOK], BF16, kind="ExternalInput")
    idn = nc.dram_tensor("idn", [128, 128], WHT, kind="ExternalInput")
    wh = nc.dram_tensor("wh", [H, H], WHT, kind="ExternalInput")
    wx = nc.dram_tensor("wx", [D, H], BF16, kind="ExternalInput")
    wo = nc.dram_tensor("wo", [H, O], BF16, kind="ExternalInput")
    bv = nc.dram_tensor("bv", [H], FP32, kind="ExternalInput")
    bov = nc.dram_tensor("bov", [O], FP32, kind="ExternalInput")
    yT = nc.dram_tensor("yT", [O, NTOK], FP32, kind="ExternalOutput")
    ymark = nc.dram_tensor("ymark", [128, 1], FP32, kind="Internal")

    # ---- SBUF ----
    XP = nc.alloc_sbuf_tensor("XP", [128, T * SLOT], BF16)        # 64K
    HS = nc.alloc_sbuf_tensor("HS", [128, RING * SLOT], BF16)     # 16K
    XTs = nc.alloc_sbuf_tensor("XTs", [128, KD * NTOK], BF16)     # 16K
    WhS = nc.alloc_sbuf_tensor("WhS", [128, KH * H], WHT)
    WxS = nc.alloc_sbuf_tensor("WxS", [128, KD * H], BF16)
    WoS = nc.alloc_sbuf_tensor("WoS", [128, KH * O], BF16)
    bS = nc.alloc_sbuf_tensor("bS", [128, MH], FP32)
    boS = nc.alloc_sbuf_tensor("boS", [128, 1], FP32)
    IdS = nc.alloc_sbuf_tensor("IdS", [128, 128], WHT)
    YTs = nc.alloc_sbuf_tensor("YTs", [128, 2 * NT], FP32)        # 4K

    # ---- PSUM: 4x phase-1/3 tiles + 2x2 step groups = 8 banks ----
    P1 = [nc.alloc_psum_tensor(f"P1_{i}", [128, NT], FP32) for i in range(4)]
    PA = [nc.alloc_psum_tensor(f"PA_{i}", [128, SLOT // 2], FP32) for i in range(2)]
    PB = [nc.alloc_psum_tensor(f"PB_{i}", [128, SLOT // 2], FP32) for i in range(2)]

    # ---- semaphores ----
    W1 = nc.alloc_semaphore("w1_v7")    # phase-1 inputs loaded
    WS = nc.alloc_semaphore("ws_v7")    # everything loaded
    P1S = nc.alloc_semaphore("p1s_v7")
    A1S = nc.alloc_semaphore("a1s_v7")
    A1V = nc.alloc_semaphore("a1v_v7")
    ZA = nc.alloc_semaphore("za_v7")
    ZB = nc.alloc_semaphore("zb_v7")
    TA = nc.alloc_semaphore("ta_v7")
    TB = nc.alloc_semaphore("tb_v7")
    P3S = nc.alloc_semaphore("p3s_v7")
    YAS = nc.alloc_semaphore("yas_v7")
    YDS = nc.alloc_semaphore("yds_v7")
    YJS = nc.alloc_semaphore("yjs_v7")

    for s in (W1, WS, P1S, A1S, A1V, ZA, ZB, TA, TB, P3S, YAS, YDS, YJS):
        nc.gpsimd.sem_clear(s)
    nc.all_engine_barrier()

    # ---- init loads: phase-1 inputs first so phase 1 starts early ----
    n1 = 0
    for k in range(KD):
        nc.sync.dma_start(WxS[:, k * H:(k + 1) * H],
                          wx[k * 128:(k + 1) * 128, :]).then_inc(W1, 16)
        n1 += 1
    with nc.allow_non_contiguous_dma(reason="tiny bias vector"):
        nc.sync.dma_start(bS[:], bv[:].rearrange("(m p) -> p m", p=128)).then_inc(W1, 16)
    n1 += 1
    for k in range(KD):
        nc.sync.dma_start(XTs[:, k * NTOK:(k + 1) * NTOK],
                          xT[k * 128:(k + 1) * 128, :]).then_inc(W1, 16)
        n1 += 1
    W1TARGET = 16 * n1

    nw = 0
    nc.sync.dma_start(IdS[:], idn[:, :]).then_inc(WS, 16)
    nw += 1
    for k in range(KH):
        nc.sync.dma_start(WhS[:, k * H:(k + 1) * H],
                          wh[k * 128:(k + 1) * 128, :]).then_inc(WS, 16)
        nw += 1
    for k in range(KH):
        nc.sync.dma_start(WoS[:, k * O:(k + 1) * O],
                          wo[k * 128:(k + 1) * 128, :]).then_inc(WS, 16)
        nw += 1
    with nc.allow_non_contiguous_dma(reason="tiny bias vector"):
        nc.sync.dma_start(boS[:], bov[:].rearrange("(p one) -> p one", one=1)).then_inc(WS, 16)
    nw += 1
    nc.vector.memset(HS[:, (RING - 1) * SLOT:RING * SLOT], 0.0).then_inc(WS, 16)
    nw += 1
    WTARGET = 16 * nw

    nc.tensor.wait_ge(W1, W1TARGET)
    nc.scalar.wait_ge(W1, W1TARGET)
    nc.vector.wait_ge(W1, W1TARGET)

    XP3 = XP[:].rearrange("p (s f) -> p s f", f=SLOT)    # [128, T, 64]
    HSr = HS[:].rearrange("p (r s) -> p r s", s=SLOT)    # [128, RING, 64]

    # ---- phase 1: s*xproj^T = (s*Wx)^T x^T + s*b ----
    NPAIR = (NTOK // NT) * MH
    pair = 0
    for nt in range(NTOK // NT):
        for m in range(MH):
            q = pair % 4
            if pair >= 4:
                pp4 = pair - 4
                if pp4 % 2 == 0:
                    nc.tensor.wait_ge(A1S, pp4 // 2 + 1)
                else:
                    nc.tensor.wait_ge(A1V, pp4 // 2 + 1)
            for k in range(KD):
                mm = nc.tensor.matmul(
                    P1[q][:],
                    WxS[:, k * H + 128 * m: k * H + 128 * (m + 1)],
                    XTs[:, k * NTOK + nt * NT: k * NTOK + (nt + 1) * NT],
                    start=(k == 0),
                    stop=(k == KD - 1),
                )
                if k == KD - 1:
                    mm.then_inc(P1S, 1)
            pair += 1
    # act/vector side of phase 1
    pair = 0
    for nt in range(NTOK // NT):
        for m in range(MH):
            q = pair % 4
            dst = XP3[:, nt * 64:(nt + 1) * 64, 8 * m: 8 * (m + 1)]
            if pair % 2 == 0:
                nc.scalar.wait_ge(P1S, pair + 1)
                nc.scalar.activation(
                    dst, P1[q][:], mybir.ActivationFunctionType.Identity,
                    bias=bS[:, m:m + 1],
                ).then_inc(A1S, 1)
            else:
                nc.vector.wait_ge(P1S, pair + 1)
                nc.vector.tensor_scalar_add(
                    dst, P1[q][:], bS[:, m:m + 1],
                ).then_inc(A1V, 1)
            pair += 1

    # all XP written + remaining weights loaded before the recurrence
    nc.tensor.wait_ge(A1S, NPAIR // 2)
    nc.tensor.wait_ge(A1V, NPAIR // 2)
    nc.tensor.wait_ge(WS, WTARGET)
    nc.scalar.wait_ge(WS, WTARGET)

    # ---- recurrence, fully unrolled ----
    inv_s = 1.0 / SCALE if WH_FP8 else 1.0

    for blk in range(NBLK):
        for j in range(64):
            t = blk * 64 + j
            p = t % 2
            hp = ((t - 1) % RING) * SLOT      # hprev slot offset
            hc = (t % RING) * SLOT            # hcur slot offset
            xo = t * SLOT
            # PSUM WAR: tanh of step t-2 (same parity) must be done
            if t >= 2:
                nc.tensor.wait_ge(TA, t - 1)
            nc.tensor.matmul(PA[p][:], IdS[:], XP[:, xo:xo + 32],
                             start=True, stop=False, skip_group_check=True)
            if t >= 2:
                nc.tensor.wait_ge(TB, t - 1)
            nc.tensor.matmul(PB[p][:], IdS[:], XP[:, xo + 32:xo + 64],
                             start=True, stop=False, skip_group_check=True)
            if t >= 1:
                nc.tensor.wait_ge(TA, t)
            for k in range(KH):
                if k == 4 and t >= 1:
                    nc.tensor.wait_ge(TB, t)
                for m in range(4):
                    mm = nc.tensor.matmul(
                        PA[p][:, 8 * m: 8 * (m + 1)],
                        WhS[:, k * H + 128 * m: k * H + 128 * (m + 1)],
                        HS[:, hp + k * 8: hp + (k + 1) * 8],
                        start=False, stop=(k == KH - 1),
                        skip_group_check=True,
                    )
                    if k == KH - 1 and m == 3:
                        mm.then_inc(ZA, 1)
            for k in range(KH):
                for m in range(4, 8):
                    mm = nc.tensor.matmul(
                        PB[p][:, 8 * (m - 4): 8 * (m - 3)],
                        WhS[:, k * H + 128 * m: k * H + 128 * (m + 1)],
                        HS[:, hp + k * 8: hp + (k + 1) * 8],
                        start=False, stop=(k == KH - 1),
                        skip_group_check=True,
                    )
                    if k == KH - 1 and m == 7:
                        mm.then_inc(ZB, 1)
            # Act: tanh halves straight into the ring
            nc.scalar.wait_ge(ZA, t + 1)
            nc.scalar.activation(HS[:, hc:hc + 32], PA[p][:],
                                 mybir.ActivationFunctionType.Tanh,
                                 scale=inv_s).then_inc(TA, 1)
            nc.scalar.wait_ge(ZB, t + 1)
            nc.scalar.activation(HS[:, hc + 32:hc + 64], PB[p][:],
                                 mybir.ActivationFunctionType.Tanh,
                                 scale=inv_s).then_inc(TB, 1)

        # ---- y for this block ----
        pp = blk % 2
        nc.tensor.wait_ge(TB, (blk + 1) * 64)
        if blk >= 2:
            nc.tensor.wait_ge(YAS, blk - 1)   # P1[pp] free
        for k in range(KH):
            mm = nc.tensor.matmul(
                P1[pp][:],
                WoS[:, k * O:(k + 1) * O],
                HSr[:, pp * 64:(pp + 1) * 64, k * 8:(k + 1) * 8],
                start=(k == 0), stop=(k == KH - 1),
            )
            if k == KH - 1:
                mm.then_inc(P3S, 1)
        nc.scalar.wait_ge(P3S, blk + 1)
        if blk >= 2:
            # YTs[pp] free once block blk-2's y DMA landed; wait for all
            # markers through blk-1 (marker blk-1 <- act blk-1 which precedes
            # this act on the same engine, so never circular)
            nc.scalar.wait_ge(YDS, 16 * blk)
        nc.scalar.activation(YTs[:, pp * NT:(pp + 1) * NT], P1[pp][:],
                             mybir.ActivationFunctionType.Identity,
                             bias=boS[:]).then_inc(YAS, 1)
        nc.sync.wait_ge(YAS, blk + 1)
        nc.sync.dma_start(yT[:, blk * NT:(blk + 1) * NT],
                          YTs[:, pp * NT:(pp + 1) * NT]).then_inc(YJS, 16)
        nc.sync.dma_start(ymark[:, :], YTs[:, pp * NT:pp * NT + 1]) \
            .then_inc(YDS, 16)

    nc.gpsimd.wait_ge(YDS, 16 * NBLK)
    nc.compile()
    return nc


def _get_nc(T=512):
    if T not in _cached:
        _cached[T] = _build(T)
    return _cached[T]


def make_inputs(inputs, Wx, Wh, b, Wo, bo, T=512):
    """Host-side shard + quantize; returns per-core input maps."""
    NTOK = BL * T
    x = np.asarray(inputs, dtype=np.float32)
    xT_full = np.ascontiguousarray(x.transpose(2, 1, 0)).astype(bfnp)  # [D,T,B]
    s = SCALE if WH_FP8 else 1.0
    if WH_FP8:
        whq = (np.asarray(Wh, np.float32) * s).astype(e3np)
        idq = np.eye(128, dtype=np.float32).astype(e3np)
    else:
        whq = np.asarray(Wh, np.float32).astype(bfnp)
        idq = np.eye(128, dtype=np.float32).astype(bfnp)
    wxb = (np.asarray(Wx, np.float32) * s).astype(bfnp)
    bf = np.ascontiguousarray(np.asarray(b, np.float32) * s)
    wob = np.asarray(Wo, np.float32).astype(bfnp)
    bof = np.ascontiguousarray(np.asarray(bo, np.float32))
    in_maps = []
    for c in range(NCORES):
        xs = np.ascontiguousarray(xT_full[:, :T, c * BL:(c + 1) * BL]).reshape(D, NTOK)
        in_maps.append({
            "xT": xs, "wh": whq, "wx": wxb, "wo": wob, "bv": bf, "bov": bof,
            "idn": idq,
        })
    return in_maps


def kernel(inputs, Wx, Wh, b, Wo, bo):
    global LAST_RESULTS
    T = 512
    nc = _get_nc(T)
    in_maps = make_inputs(inputs, Wx, Wh, b, Wo, bo, T)
    res = run_bass_kernel_spmd(nc, in_maps, list(range(NCORES)))
    LAST_RESULTS = res
    y = np.empty((B, T, O), np.float32)
    for c in range(NCORES):
        ytc = res.results[c]["yT"]
        y[c * BL:(c + 1) * BL] = ytc.reshape(O, T, BL).transpose(2, 1, 0)
    return y


# revision 3
# speedup vs baseline: 1.0158x; 1.0158x over previous
"""Trainium2 Bass kernel for the Keras SimpleRNN wrapper — v9.

    xproj = inputs @ (s*Wx) + s*b              # [B, T, H]  (held as s*xproj)
    h_t   = tanh((xproj_t + h_{t-1} @ (s*Wh)) / s)
    y     = h @ Wo + bo                        # [B, T, O]

Data-parallel over batch (8 rows/core). v3 changes vs v2:
  - FULL UNROLL: no Fori hardware loops -> pure sequential instruction
    streams (no backward-branch instruction-queue refetch stalls), and
    every AP/semaphore wait is a static constant.
  - tanh writes h directly into a 128-slot SBUF ring (HS); the y matmuls
    read the ring with strided APs. The per-step DVE copy, the CS
    semaphore and the per-block PE barrier on DVE are gone.
  - phase 1 uses a 4-deep PSUM ring with bias-adds alternating between
    the Scalar (Act) and Vector engines, so the PE never waits for PSUM.
  - optional fp8e3 (e3m4) Wh with power-of-2 prescale s folded into
    Wx/b on the host and tanh(in * 1/s) on the Act; fp8 halves the
    LDWEIGHTS time per step matmul (FWL is byte-bound).
"""

import numpy as np
import ml_dtypes

import concourse.bass as bass
import concourse.mybir as mybir
from concourse import bacc
from concourse.bass_utils import run_bass_kernel_spmd

BF16 = mybir.dt.bfloat16
FP32 = mybir.dt.float32
E3 = mybir.dt.float8e3
bfnp = ml_dtypes.bfloat16
e3np = ml_dtypes.float8_e3m4

B, D, H, O = 64, 256, 1024, 128
NCORES = 8
BL = B // NCORES          # 8 batch rows per core
SLOT = 8 * BL             # 64 cols per timestep slot
KH = H // 128             # 8 contraction chunks for Wh/Wo
MH = H // 128
KD = D // 128             # 2 contraction chunks for Wx
RING = 128                # h-ring slots (2 blocks)

# --- tunables (set before first _get_nc call) ---
WH_FP8 = False            # fp8 is a dead end: LDWEIGHTS is col-count-bound
SCALE = 16.0              # power-of-2 prescale (only used when WH_FP8)

_cached = {}
LAST_RESULTS = None


def _build(T=512):
    NTOK = BL * T
    NT = 512              # tokens per phase-1/3 tile (64 steps x 8 batch)
    NBLK = T // 64
    WHT = E3 if WH_FP8 else BF16

    nc = bacc.Bacc("TRN2", target_bir_lowering=False, debug=False)

    xT = nc.dram_tensor("xT", [D, NTOK], BF16, kind="ExternalInput")
    idn = nc.dram_tensor("idn", [128, 128], WHT, kind="ExternalInput")
    wh = nc.dram_tensor("wh", [H, H], WHT, kind="ExternalInput")
    wx = nc.dram_tensor("wx", [D, H], BF16, kind="ExternalInput")
    wo = nc.dram_tensor("wo", [H, O], BF16, kind="ExternalInput")
    bv = nc.dram_tensor("bv", [H], FP32, kind="ExternalInput")
    bov = nc.dram_tensor("bov", [O], FP32, kind="ExternalInput")
    yT = nc.dram_tensor("yT", [O, NTOK], FP32, kind="ExternalOutput")
    ymark = nc.dram_tensor("ymark", [128, 1], FP32, kind="Internal")

    # ---- SBUF ----
    XP = nc.alloc_sbuf_tensor("XP", [128, T * SLOT], BF16)        # 64K
    HS = nc.alloc_sbuf_tensor("HS", [128, RING * SLOT], BF16)     # 16K
    XTs = nc.alloc_sbuf_tensor("XTs", [128, KD * NTOK], BF16)     # 16K
    WhS = nc.alloc_sbuf_tensor("WhS", [128, KH * H], WHT)
    WxS = nc.alloc_sbuf_tensor("WxS", [128, KD * H], BF16)
    WoS = nc.alloc_sbuf_tensor("WoS", [128, KH * O], BF16)
    bS = nc.alloc_sbuf_tensor("bS", [128, MH], FP32)
    boS = nc.alloc_sbuf_tensor("boS", [128, 1], FP32)
    IdS = nc.alloc_sbuf_tensor("IdS", [128, 128], WHT)
    YTs = nc.alloc_sbuf_tensor("YTs", [128, 2 * NT], FP32)        # 4K

    # ---- PSUM: 4x phase-1/3 tiles + 2x2 step groups = 8 banks ----
    P1 = [nc.alloc_psum_tensor(f"P1_{i}", [128, NT], FP32) for i in range(4)]
    PA = [nc.alloc_psum_tensor(f"PA_{i}", [128, SLOT // 2], FP32) for i in range(2)]
    PB = [nc.alloc_psum_tensor(f"PB_{i}", [128, SLOT // 2], FP32) for i in range(2)]

    # ---- semaphores ----
    W1 = nc.alloc_semaphore("w1_v7")    # phase-1 weights/bias loaded
    XS = nc.alloc_semaphore("xs_v9")    # DVE xproj->PSUM copies   +1/step (t>=2)
    WX = nc.alloc_semaphore("wx_v8")    # x^T loaded (gpsimd queue)
    WS = nc.alloc_semaphore("ws_v7")    # everything loaded
    P1S = nc.alloc_semaphore("p1s_v7")
    A1S = nc.alloc_semaphore("a1s_v7")
    A1V = nc.alloc_semaphore("a1v_v7")
    ZA = nc.alloc_semaphore("za_v7")
    ZB = nc.alloc_semaphore("zb_v7")
    TA = nc.alloc_semaphore("ta_v7")
    TB = nc.alloc_semaphore("tb_v7")
    P3S = nc.alloc_semaphore("p3s_v7")
    YAS = nc.alloc_semaphore("yas_v7")
    YDS = nc.alloc_semaphore("yds_v7")
    YJS = nc.alloc_semaphore("yjs_v7")

    for s in (W1, WX, WS, P1S, A1S, A1V, XS, ZA, ZB, TA, TB, P3S, YAS, YDS, YJS):
        nc.gpsimd.sem_clear(s)
    nc.all_engine_barrier()

    # ---- init loads: phase-1 inputs first so phase 1 starts early ----
    n1 = 0
    for k in range(KD):
        nc.sync.dma_start(WxS[:, k * H:(k + 1) * H],
                          wx[k * 128:(k + 1) * 128, :]).then_inc(W1, 16)
        n1 += 1
    with nc.allow_non_contiguous_dma(reason="tiny bias vector"):
        nc.sync.dma_start(bS[:], bv[:].rearrange("(m p) -> p m", p=128)).then_inc(W1, 16)
    n1 += 1
    for k in range(KD):
        # separate queue: overlaps with the sync-queue weight loads
        nc.gpsimd.dma_start(XTs[:, k * NTOK:(k + 1) * NTOK],
                            xT[k * 128:(k + 1) * 128, :]).then_inc(WX, 16)
    W1TARGET = 16 * n1

    nw = 0
    nc.sync.dma_start(IdS[:], idn[:, :]).then_inc(WS, 16)
    nw += 1
    for k in range(KH):
        nc.sync.dma_start(WhS[:, k * H:(k + 1) * H],
                          wh[k * 128:(k + 1) * 128, :]).then_inc(WS, 16)
        nw += 1
    for k in range(KH):
        nc.sync.dma_start(WoS[:, k * O:(k + 1) * O],
                          wo[k * 128:(k + 1) * 128, :]).then_inc(WS, 16)
        nw += 1
    with nc.allow_non_contiguous_dma(reason="tiny bias vector"):
        nc.sync.dma_start(boS[:], bov[:].rearrange("(p one) -> p one", one=1)).then_inc(WS, 16)
    nw += 1
    nc.vector.memset(HS[:, (RING - 1) * SLOT:RING * SLOT], 0.0).then_inc(WS, 16)
    nw += 1
    WTARGET = 16 * nw

    nc.tensor.wait_ge(W1, W1TARGET)
    nc.tensor.wait_ge(WX, 32)
    nc.scalar.wait_ge(W1, W1TARGET)
    nc.vector.wait_ge(W1, W1TARGET)

    XP3 = XP[:].rearrange("p (s f) -> p s f", f=SLOT)    # [128, T, 64]
    HSr = HS[:].rearrange("p (r s) -> p r s", s=SLOT)    # [128, RING, 64]

    # ---- phase 1: s*xproj^T = (s*Wx)^T x^T + s*b ----
    NPAIR = (NTOK // NT) * MH
    pair = 0
    for nt in range(NTOK // NT):
        for m in range(MH):
            q = pair % 4
            if pair >= 4:
                pp4 = pair - 4
                if pp4 % 2 == 0:
                    nc.tensor.wait_ge(A1S, pp4 // 2 + 1)
                else:
                    nc.tensor.wait_ge(A1V, pp4 // 2 + 1)
            for k in range(KD):
                mm = nc.tensor.matmul(
                    P1[q][:],
                    WxS[:, k * H + 128 * m: k * H + 128 * (m + 1)],
                    XTs[:, k * NTOK + nt * NT: k * NTOK + (nt + 1) * NT],
                    start=(k == 0),
                    stop=(k == KD - 1),
                )
                if k == KD - 1:
                    mm.then_inc(P1S, 1)
            pair += 1
    # act/vector side of phase 1
    pair = 0
    for nt in range(NTOK // NT):
        for m in range(MH):
            q = pair % 4
            dst = XP3[:, nt * 64:(nt + 1) * 64, 8 * m: 8 * (m + 1)]
            if pair % 2 == 0:
                nc.scalar.wait_ge(P1S, pair + 1)
                nc.scalar.activation(
                    dst, P1[q][:], mybir.ActivationFunctionType.Identity,
                    bias=bS[:, m:m + 1],
                ).then_inc(A1S, 1)
            else:
                nc.vector.wait_ge(P1S, pair + 1)
                nc.vector.tensor_scalar_add(
                    dst, P1[q][:], bS[:, m:m + 1],
                ).then_inc(A1V, 1)
            pair += 1

    # all XP written + remaining weights loaded before the recurrence
    nc.tensor.wait_ge(A1S, NPAIR // 2)
    nc.tensor.wait_ge(A1V, NPAIR // 2)
    nc.tensor.wait_ge(WS, WTARGET)
    nc.scalar.wait_ge(WS, WTARGET)

    # ---- recurrence, fully unrolled ----
    inv_s = 1.0 / SCALE if WH_FP8 else 1.0

    for blk in range(NBLK):
        for j in range(64):
            t = blk * 64 + j
            p = t % 2
            hp = ((t - 1) % RING) * SLOT      # hprev slot offset
            hc = (t % RING) * SLOT            # hcur slot offset
            xo = t * SLOT
            # TA>=t covers both the PA WAR of step t-2 (TA counts through
            # t-1 >= t-2+1) and the A-half reads of h_{t-1}; ditto TB.
            # t<2: identity-MM epochs (start=True) prime the PSUM banks'
            # has_written bits; t>=2: the DVE pre-writes xproj and the
            # Wh matmuls accumulate straight onto it.
            if t >= 2:
                nc.tensor.wait_ge(XS, t - 1)
            if t >= 1:
                nc.tensor.wait_ge(TA, t)
            if t < 2:
                nc.tensor.matmul(PA[p][:], IdS[:], XP[:, xo:xo + 32],
                                 start=True, stop=False, skip_group_check=True)
            for k in range(4):
                for m in range(4):
                    nc.tensor.matmul(
                        PA[p][:, 8 * m: 8 * (m + 1)],
                        WhS[:, k * H + 128 * m: k * H + 128 * (m + 1)],
                        HS[:, hp + k * 8: hp + (k + 1) * 8],
                        start=False, stop=False,
                        skip_group_check=True,
                    )
            if t >= 1:
                nc.tensor.wait_ge(TB, t)
            if t < 2:
                nc.tensor.matmul(PB[p][:], IdS[:], XP[:, xo + 32:xo + 64],
                                 start=True, stop=False, skip_group_check=True)
            for k in range(4, 8):
                for m in range(4):
                    mm = nc.tensor.matmul(
                        PA[p][:, 8 * m: 8 * (m + 1)],
                        WhS[:, k * H + 128 * m: k * H + 128 * (m + 1)],
                        HS[:, hp + k * 8: hp + (k + 1) * 8],
                        start=False, stop=(k == KH - 1),
                        skip_group_check=True,
                    )
                    if k == KH - 1 and m == 3:
                        mm.then_inc(ZA, 1)
            for k in range(KH):
                for m in range(4, 8):
                    mm = nc.tensor.matmul(
                        PB[p][:, 8 * (m - 4): 8 * (m - 3)],
                        WhS[:, k * H + 128 * m: k * H + 128 * (m + 1)],
                        HS[:, hp + k * 8: hp + (k + 1) * 8],
                        start=False, stop=(k == KH - 1),
                        skip_group_check=True,
                    )
                    if k == KH - 1 and m == 7:
                        mm.then_inc(ZB, 1)
            # Act: tanh halves straight into the ring
            nc.scalar.wait_ge(ZA, t + 1)
            nc.scalar.activation(HS[:, hc:hc + 32], PA[p][:],
                                 mybir.ActivationFunctionType.Tanh,
                                 scale=inv_s).then_inc(TA, 1)
            nc.scalar.wait_ge(ZB, t + 1)
            nc.scalar.activation(HS[:, hc + 32:hc + 64], PB[p][:],
                                 mybir.ActivationFunctionType.Tanh,
                                 scale=inv_s).then_inc(TB, 1)

        # ---- y for this block ----
        pp = blk % 2
        nc.tensor.wait_ge(TB, (blk + 1) * 64)
        if blk >= 2:
            nc.tensor.wait_ge(YAS, blk - 1)   # P1[pp] free
        for k in range(KH):
            mm = nc.tensor.matmul(
                P1[pp][:],
                WoS[:, k * O:(k + 1) * O],
                HSr[:, pp * 64:(pp + 1) * 64, k * 8:(k + 1) * 8],
                start=(k == 0), stop=(k == KH - 1),
            )
            if k == KH - 1:
                mm.then_inc(P3S, 1)
        nc.scalar.wait_ge(P3S, blk + 1)
        if blk >= 2:
            # YTs[pp] free once block blk-2's y DMA landed; wait for all
            # markers through blk-1 (marker blk-1 <- act blk-1 which precedes
            # this act on the same engine, so never circular)
            nc.scalar.wait_ge(YDS, 16 * blk)
        nc.scalar.activation(YTs[:, pp * NT:(pp + 1) * NT], P1[pp][:],
                             mybir.ActivationFunctionType.Identity,
                             bias=boS[:]).then_inc(YAS, 1)
        nc.sync.wait_ge(YAS, blk + 1)
        nc.sync.dma_start(yT[:, blk * NT:(blk + 1) * NT],
                          YTs[:, pp * NT:(pp + 1) * NT]).then_inc(YJS, 16)
        nc.sync.dma_start(ymark[:, :], YTs[:, pp * NT:pp * NT + 1]) \
            .then_inc(YDS, 16)

    # ---- DVE stream: xproj -> PSUM for steps t>=2 ----
    for t in range(2, T):
        p = t % 2
        xo = t * SLOT
        if t % 64 == 0:
            nc.vector.wait_ge(A1S, (t // 64 + 1) * 4)
        nc.vector.wait_ge(TA, t - 1)
        nc.vector.tensor_copy(PA[p][:], XP[:, xo:xo + 32])
        nc.vector.wait_ge(TB, t - 1)
        nc.vector.tensor_copy(PB[p][:], XP[:, xo + 32:xo + 64]).then_inc(XS, 1)

    nc.gpsimd.wait_ge(YDS, 16 * NBLK)
    nc.compile()
    return nc


def _get_nc(T=512):
    if T not in _cached:
        _cached[T] = _build(T)
    return _cached[T]


def make_inputs(inputs, Wx, Wh, b, Wo, bo, T=512):
    """Host-side shard + quantize; returns per-core input maps."""
    NTOK = BL * T
    x = np.asarray(inputs, dtype=np.float32)
    xT_full = np.ascontiguousarray(x.transpose(2, 1, 0)).astype(bfnp)  # [D,T,B]
    s = SCALE if WH_FP8 else 1.0
    if WH_FP8:
        whq = (np.asarray(Wh, np.float32) * s).astype(e3np)
        idq = np.eye(128, dtype=np.float32).astype(e3np)
    else:
        whq = np.asarray(Wh, np.float32).astype(bfnp)
        idq = np.eye(128, dtype=np.float32).astype(bfnp)
    wxb = (np.asarray(Wx, np.float32) * s).astype(bfnp)
    bf = np.ascontiguousarray(np.asarray(b, np.float32) * s)
    wob = np.asarray(Wo, np.float32).astype(bfnp)
    bof = np.ascontiguousarray(np.asarray(bo, np.float32))
    in_maps = []
    for c in range(NCORES):
        xs = np.ascontiguousarray(xT_full[:, :T, c * BL:(c + 1) * BL]).reshape(D, NTOK)
        in_maps.append({
            "xT": xs, "wh": whq, "wx": wxb, "wo": wob, "bv": bf, "bov": bof,
            "idn": idq,
        })
    return in_maps


def kernel(inputs, Wx, Wh, b, Wo, bo):
    global LAST_RESULTS
    T = 512
    nc = _get_nc(T)
    in_maps = make_inputs(inputs, Wx, Wh, b, Wo, bo, T)
    res = run_bass_kernel_spmd(nc, in_maps, list(range(NCORES)))
    LAST_RESULTS = res
    y = np.empty((B, T, O), np.float32)
    for c in range(NCORES):
        ytc = res.results[c]["yT"]
        y[c * BL:(c + 1) * BL] = ytc.reshape(O, T, BL).transpose(2, 1, 0)
    return y


# revision 4
# speedup vs baseline: 1.0269x; 1.0110x over previous
"""Trainium2 Bass kernel for the Keras SimpleRNN wrapper — v10.

    xproj = inputs @ (s*Wx) + s*b              # [B, T, H]  (held as s*xproj)
    h_t   = tanh((xproj_t + h_{t-1} @ (s*Wh)) / s)
    y     = h @ Wo + bo                        # [B, T, O]

Data-parallel over batch (8 rows/core). v3 changes vs v2:
  - FULL UNROLL: no Fori hardware loops -> pure sequential instruction
    streams (no backward-branch instruction-queue refetch stalls), and
    every AP/semaphore wait is a static constant.
  - tanh writes h directly into a 128-slot SBUF ring (HS); the y matmuls
    read the ring with strided APs. The per-step DVE copy, the CS
    semaphore and the per-block PE barrier on DVE are gone.
  - phase 1 uses a 4-deep PSUM ring with bias-adds alternating between
    the Scalar (Act) and Vector engines, so the PE never waits for PSUM.
  - optional fp8e3 (e3m4) Wh with power-of-2 prescale s folded into
    Wx/b on the host and tanh(in * 1/s) on the Act; fp8 halves the
    LDWEIGHTS time per step matmul (FWL is byte-bound).
"""

import numpy as np
import ml_dtypes

import concourse.bass as bass
import concourse.mybir as mybir
from concourse import bacc
from concourse.bass_utils import run_bass_kernel_spmd

BF16 = mybir.dt.bfloat16
FP32 = mybir.dt.float32
E3 = mybir.dt.float8e3
bfnp = ml_dtypes.bfloat16
e3np = ml_dtypes.float8_e3m4

B, D, H, O = 64, 256, 1024, 128
NCORES = 8
BL = B // NCORES          # 8 batch rows per core
SLOT = 8 * BL             # 64 cols per timestep slot
KH = H // 128             # 8 contraction chunks for Wh/Wo
MH = H // 128
KD = D // 128             # 2 contraction chunks for Wx
RING = 128                # h-ring slots (2 blocks)

# --- tunables (set before first _get_nc call) ---
WH_FP8 = False            # fp8 is a dead end: LDWEIGHTS is col-count-bound
SCALE = 16.0              # power-of-2 prescale (only used when WH_FP8)

_cached = {}
LAST_RESULTS = None


def _build(T=512):
    NTOK = BL * T
    NT = 512              # tokens per phase-1/3 tile (64 steps x 8 batch)
    NBLK = T // 64
    WHT = E3 if WH_FP8 else BF16

    nc = bacc.Bacc("TRN2", target_bir_lowering=False, debug=False)

    xT = nc.dram_tensor("xT", [D, NTOK], BF16, kind="ExternalInput")
    idn = nc.dram_tensor("idn", [128, 128], WHT, kind="ExternalInput")
    wh = nc.dram_tensor("wh", [H, H], WHT, kind="ExternalInput")
    wx = nc.dram_tensor("wx", [D, H], BF16, kind="ExternalInput")
    wo = nc.dram_tensor("wo", [H, O], BF16, kind="ExternalInput")
    bv = nc.dram_tensor("bv", [H], FP32, kind="ExternalInput")
    bov = nc.dram_tensor("bov", [O], FP32, kind="ExternalInput")
    yT = nc.dram_tensor("yT", [O, NTOK], FP32, kind="ExternalOutput")
    ymark = nc.dram_tensor("ymark", [128, 1], FP32, kind="Internal")

    # ---- SBUF ----
    XP = nc.alloc_sbuf_tensor("XP", [128, T * SLOT], BF16)        # 64K
    HS = nc.alloc_sbuf_tensor("HS", [128, RING * SLOT], BF16)     # 16K
    XTs = nc.alloc_sbuf_tensor("XTs", [128, KD * NTOK], BF16)     # 16K
    WhS = nc.alloc_sbuf_tensor("WhS", [128, KH * H], WHT)
    WxS = nc.alloc_sbuf_tensor("WxS", [128, KD * H], BF16)
    WoS = nc.alloc_sbuf_tensor("WoS", [128, KH * O], BF16)
    bS = nc.alloc_sbuf_tensor("bS", [128, MH], FP32)
    boS = nc.alloc_sbuf_tensor("boS", [128, 1], FP32)
    IdS = nc.alloc_sbuf_tensor("IdS", [128, 128], WHT)
    YTs = nc.alloc_sbuf_tensor("YTs", [128, 2 * NT], FP32)        # 4K

    # ---- PSUM: 4x phase-1/3 tiles + 2x2 step groups = 8 banks ----
    P1 = [nc.alloc_psum_tensor(f"P1_{i}", [128, NT], FP32) for i in range(4)]
    PA = [nc.alloc_psum_tensor(f"PA_{i}", [128, SLOT // 2], FP32) for i in range(2)]
    PB = [nc.alloc_psum_tensor(f"PB_{i}", [128, SLOT // 2], FP32) for i in range(2)]

    # ---- semaphores ----
    W1 = nc.alloc_semaphore("w1_v7")    # phase-1 weights/bias loaded
    XS = nc.alloc_semaphore("xs_v9")    # DVE xproj->PSUM copies   +1/step (t>=2)
    WX = nc.alloc_semaphore("wx_v8")    # x^T loaded (gpsimd queue)
    WS = nc.alloc_semaphore("ws_v7")    # everything loaded
    P1S = nc.alloc_semaphore("p1s_v7")
    A1S = nc.alloc_semaphore("a1s_v7")
    A1V = nc.alloc_semaphore("a1v_v7")
    ZA = nc.alloc_semaphore("za_v7")
    ZB = nc.alloc_semaphore("zb_v7")
    TA = nc.alloc_semaphore("ta_v7")
    TB = nc.alloc_semaphore("tb_v7")
    P3S = nc.alloc_semaphore("p3s_v7")
    YAS = nc.alloc_semaphore("yas_v7")
    YDS = nc.alloc_semaphore("yds_v7")
    YJS = nc.alloc_semaphore("yjs_v7")

    for s in (W1, WX, WS, P1S, A1S, A1V, XS, ZA, ZB, TA, TB, P3S, YAS, YDS, YJS):
        nc.gpsimd.sem_clear(s)
    nc.all_engine_barrier()

    # ---- init loads: phase-1 inputs first so phase 1 starts early ----
    n1 = 0
    for k in range(KD):
        nc.sync.dma_start(WxS[:, k * H:(k + 1) * H],
                          wx[k * 128:(k + 1) * 128, :]).then_inc(W1, 16)
        n1 += 1
    with nc.allow_non_contiguous_dma(reason="tiny bias vector"):
        nc.sync.dma_start(bS[:], bv[:].rearrange("(m p) -> p m", p=128)).then_inc(W1, 16)
    n1 += 1
    for k in range(KD):
        # separate HWDGE queue: overlaps with the sync-queue weight loads
        nc.scalar.dma_start(XTs[:, k * NTOK:(k + 1) * NTOK],
                            xT[k * 128:(k + 1) * 128, :]).then_inc(WX, 16)
    W1TARGET = 16 * n1

    nw = 0
    nc.sync.dma_start(IdS[:], idn[:, :]).then_inc(WS, 16)
    nw += 1
    for k in range(KH):
        nc.sync.dma_start(WhS[:, k * H:(k + 1) * H],
                          wh[k * 128:(k + 1) * 128, :]).then_inc(WS, 16)
        nw += 1
    for k in range(KH):
        nc.sync.dma_start(WoS[:, k * O:(k + 1) * O],
                          wo[k * 128:(k + 1) * 128, :]).then_inc(WS, 16)
        nw += 1
    with nc.allow_non_contiguous_dma(reason="tiny bias vector"):
        nc.sync.dma_start(boS[:], bov[:].rearrange("(p one) -> p one", one=1)).then_inc(WS, 16)
    nw += 1
    nc.vector.memset(HS[:, (RING - 1) * SLOT:RING * SLOT], 0.0).then_inc(WS, 16)
    nw += 1
    WTARGET = 16 * nw

    nc.tensor.wait_ge(W1, W1TARGET)
    nc.tensor.wait_ge(WX, 32)
    nc.scalar.wait_ge(W1, W1TARGET)
    nc.vector.wait_ge(W1, W1TARGET)

    XP3 = XP[:].rearrange("p (s f) -> p s f", f=SLOT)    # [128, T, 64]
    HSr = HS[:].rearrange("p (r s) -> p r s", s=SLOT)    # [128, RING, 64]

    # ---- phase 1: s*xproj^T = (s*Wx)^T x^T + s*b ----
    NPAIR = (NTOK // NT) * MH
    pair = 0
    for nt in range(NTOK // NT):
        for m in range(MH):
            q = pair % 4
            if pair >= 4:
                pp4 = pair - 4
                if pp4 % 2 == 0:
                    nc.tensor.wait_ge(A1S, pp4 // 2 + 1)
                else:
                    nc.tensor.wait_ge(A1V, pp4 // 2 + 1)
            for k in range(KD):
                mm = nc.tensor.matmul(
                    P1[q][:],
                    WxS[:, k * H + 128 * m: k * H + 128 * (m + 1)],
                    XTs[:, k * NTOK + nt * NT: k * NTOK + (nt + 1) * NT],
                    start=(k == 0),
                    stop=(k == KD - 1),
                )
                if k == KD - 1:
                    mm.then_inc(P1S, 1)
            pair += 1
    # act/vector side of phase 1
    pair = 0
    for nt in range(NTOK // NT):
        for m in range(MH):
            q = pair % 4
            dst = XP3[:, nt * 64:(nt + 1) * 64, 8 * m: 8 * (m + 1)]
            if pair % 2 == 0:
                nc.scalar.wait_ge(P1S, pair + 1)
                nc.scalar.activation(
                    dst, P1[q][:], mybir.ActivationFunctionType.Identity,
                    bias=bS[:, m:m + 1],
                ).then_inc(A1S, 1)
            else:
                nc.vector.wait_ge(P1S, pair + 1)
                nc.vector.tensor_scalar_add(
                    dst, P1[q][:], bS[:, m:m + 1],
                ).then_inc(A1V, 1)
            pair += 1

    # all XP written + remaining weights loaded before the recurrence
    nc.tensor.wait_ge(A1S, NPAIR // 2)
    nc.tensor.wait_ge(A1V, NPAIR // 2)
    nc.tensor.wait_ge(WS, WTARGET)
    nc.scalar.wait_ge(WS, WTARGET)

    # ---- recurrence, fully unrolled ----
    inv_s = 1.0 / SCALE if WH_FP8 else 1.0

    for blk in range(NBLK):
        for j in range(64):
            t = blk * 64 + j
            p = t % 2
            hp = ((t - 1) % RING) * SLOT      # hprev slot offset
            hc = (t % RING) * SLOT            # hcur slot offset
            xo = t * SLOT
            # TA>=t covers both the PA WAR of step t-2 (TA counts through
            # t-1 >= t-2+1) and the A-half reads of h_{t-1}; ditto TB.
            # t<2: identity-MM epochs (start=True) prime the PSUM banks'
            # has_written bits; t>=2: the DVE pre-writes xproj and the
            # Wh matmuls accumulate straight onto it.
            if t >= 2:
                nc.tensor.wait_ge(XS, t - 1)   # implies TA >= t
            elif t >= 1:
                nc.tensor.wait_ge(TA, t)
            if t < 2:
                nc.tensor.matmul(PA[p][:], IdS[:], XP[:, xo:xo + 32],
                                 start=True, stop=False, skip_group_check=True)
            for k in range(4):
                for m in range(4):
                    nc.tensor.matmul(
                        PA[p][:, 8 * m: 8 * (m + 1)],
                        WhS[:, k * H + 128 * m: k * H + 128 * (m + 1)],
                        HS[:, hp + k * 8: hp + (k + 1) * 8],
                        start=False, stop=False,
                        skip_group_check=True,
                    )
            if t >= 1:
                nc.tensor.wait_ge(TB, t)
            if t < 2:
                nc.tensor.matmul(PB[p][:], IdS[:], XP[:, xo + 32:xo + 64],
                                 start=True, stop=False, skip_group_check=True)
            for k in range(4, 8):
                for m in range(4):
                    mm = nc.tensor.matmul(
                        PA[p][:, 8 * m: 8 * (m + 1)],
                        WhS[:, k * H + 128 * m: k * H + 128 * (m + 1)],
                        HS[:, hp + k * 8: hp + (k + 1) * 8],
                        start=False, stop=(k == KH - 1),
                        skip_group_check=True,
                    )
                    if k == KH - 1 and m == 3:
                        mm.then_inc(ZA, 1)
            for k in range(KH):
                for m in range(4, 8):
                    mm = nc.tensor.matmul(
                        PB[p][:, 8 * (m - 4): 8 * (m - 3)],
                        WhS[:, k * H + 128 * m: k * H + 128 * (m + 1)],
                        HS[:, hp + k * 8: hp + (k + 1) * 8],
                        start=False, stop=(k == KH - 1),
                        skip_group_check=True,
                    )
                    if k == KH - 1 and m == 7:
                        mm.then_inc(ZB, 1)
            # Act: tanh halves straight into the ring
            nc.scalar.wait_ge(ZA, t + 1)
            nc.scalar.activation(HS[:, hc:hc + 32], PA[p][:],
                                 mybir.ActivationFunctionType.Tanh,
                                 scale=inv_s).then_inc(TA, 1)
            nc.scalar.wait_ge(ZB, t + 1)
            nc.scalar.activation(HS[:, hc + 32:hc + 64], PB[p][:],
                                 mybir.ActivationFunctionType.Tanh,
                                 scale=inv_s).then_inc(TB, 1)

        # ---- y for this block ----
        pp = blk % 2
        nc.tensor.wait_ge(TB, (blk + 1) * 64)
        if blk >= 2:
            nc.tensor.wait_ge(YAS, blk - 1)   # P1[pp] free
        for k in range(KH):
            mm = nc.tensor.matmul(
                P1[pp][:],
                WoS[:, k * O:(k + 1) * O],
                HSr[:, pp * 64:(pp + 1) * 64, k * 8:(k + 1) * 8],
                start=(k == 0), stop=(k == KH - 1),
            )
            if k == KH - 1:
                mm.then_inc(P3S, 1)
        nc.scalar.wait_ge(P3S, blk + 1)
        if blk >= 2:
            # YTs[pp] free once block blk-2's y DMA landed; wait for all
            # markers through blk-1 (marker blk-1 <- act blk-1 which precedes
            # this act on the same engine, so never circular)
            nc.scalar.wait_ge(YDS, 16 * blk)
        nc.scalar.activation(YTs[:, pp * NT:(pp + 1) * NT], P1[pp][:],
                             mybir.ActivationFunctionType.Identity,
                             bias=boS[:]).then_inc(YAS, 1)
        nc.sync.wait_ge(YAS, blk + 1)
        nc.sync.dma_start(yT[:, blk * NT:(blk + 1) * NT],
                          YTs[:, pp * NT:(pp + 1) * NT]).then_inc(YJS, 16)
        if blk < NBLK - 1:
            # marker lands strictly after the block's y data (same queue);
            # the last block's drain uses the data ticks directly
            nc.sync.dma_start(ymark[:, :], YTs[:, pp * NT:pp * NT + 1]) \
                .then_inc(YDS, 16)

    # ---- DVE stream: xproj -> PSUM for steps t>=2 ----
    for t in range(2, T):
        p = t % 2
        xo = t * SLOT
        if t % 64 == 0:
            nc.vector.wait_ge(A1S, (t // 64 + 1) * 4)
        # TA>=t (vs the minimal t-1): makes XS>=t-1 imply TA>=t for the PE
        nc.vector.wait_ge(TA, t)
        nc.vector.tensor_copy(PA[p][:], XP[:, xo:xo + 32])
        nc.vector.wait_ge(TB, t - 1)
        nc.vector.tensor_copy(PB[p][:], XP[:, xo + 32:xo + 64]).then_inc(XS, 1)

    nc.gpsimd.wait_ge(YDS, 16 * (NBLK - 1))
    nc.gpsimd.wait_ge(YJS, 16 * NBLK)
    nc.compile()
    return nc


def _get_nc(T=512):
    if T not in _cached:
        _cached[T] = _build(T)
    return _cached[T]


def make_inputs(inputs, Wx, Wh, b, Wo, bo, T=512):
    """Host-side shard + quantize; returns per-core input maps."""
    NTOK = BL * T
    x = np.asarray(inputs, dtype=np.float32)
    xT_full = np.ascontiguousarray(x.transpose(2, 1, 0)).astype(bfnp)  # [D,T,B]
    s = SCALE if WH_FP8 else 1.0
    if WH_FP8:
        whq = (np.asarray(Wh, np.float32) * s).astype(e3np)
        idq = np.eye(128, dtype=np.float32).astype(e3np)
    else:
        whq = np.asarray(Wh, np.float32).astype(bfnp)
        idq = np.eye(128, dtype=np.float32).astype(bfnp)
    wxb = (np.asarray(Wx, np.float32) * s).astype(bfnp)
    bf = np.ascontiguousarray(np.asarray(b, np.float32) * s)
    wob = np.asarray(Wo, np.float32).astype(bfnp)
    bof = np.ascontiguousarray(np.asarray(bo, np.float32))
    in_maps = []
    for c in range(NCORES):
        xs = np.ascontiguousarray(xT_full[:, :T, c * BL:(c + 1) * BL]).reshape(D, NTOK)
        in_maps.append({
            "xT": xs, "wh": whq, "wx": wxb, "wo": wob, "bv": bf, "bov": bof,
            "idn": idq,
        })
    return in_maps


def kernel(inputs, Wx, Wh, b, Wo, bo):
    global LAST_RESULTS
    T = 512
    nc = _get_nc(T)
    in_maps = make_inputs(inputs, Wx, Wh, b, Wo, bo, T)
    res = run_bass_kernel_spmd(nc, in_maps, list(range(NCORES)))
    LAST_RESULTS = res
    y = np.empty((B, T, O), np.float32)
    for c in range(NCORES):
        ytc = res.results[c]["yT"]
        y[c * BL:(c + 1) * BL] = ytc.reshape(O, T, BL).transpose(2, 1, 0)
    return y
